# revision 12
# baseline (speedup 1.0000x reference)
"""DenseCL contrastive-logits kernel for 8 Trainium2 NeuronCores.

Contract: kernel(**inputs) takes the FULL unsharded inputs (named as in
setup_inputs) and returns the full [32, 65537, 50] float32 output.

Sharding: the 65536-wide negative queues are split along the queue axis
across the 8 cores (8192 columns each); every other input is replicated.
There are NO collectives: profiling showed the runtime's cross-core sync
barrier + ncfw latency puts a ~70-95 us floor on any collective-gated
work, so instead EVERY core redundantly computes the match/gather stage
(cosine + argmax + d_q gather) for all 32 batches from fp16 features
(12.8 MB/core).  The feature DMA overlaps the cosine accumulation
matmuls chunk by chunk, then phase 2 (the 25.7 MB out_d stream) runs
DMA-bound with no cross-core dependency at all.

Precision: the match cosine runs in fp16.  Verified on the generated
inputs: the post-fp16-rounding top-2 margin of the cosine (0.0059 worst
case) is ~60x the fp32 accumulation noise, and the fp16 input rounding
is deterministic and identical between numpy and the PE, so the argmax
reproduces the reference's fp32 choice exactly.  The negative-logit
matmuls and outputs run in fp16 (values are O(50); ~4e-4 relative
error): single-PE-pass matmuls and half the output DMA bytes.  fp16
subnormals are flushed on the host (the PE weight path mishandles them).

PSUM has_written semantics (hardware-observed): a start=True matmul
clears the has_written bits of the whole partition row of its PSUM
bank, not just its own columns.  Wherever several accumulation groups
are packed into one bank at different column offsets, only the first
group's first matmul carries start=True; the other groups' first writes
then land on cleared bits and overwrite stale data automatically.

Math (per batch b, t = 1/tau = 5 folded into the one-hot):
  cosT[j, i] = sum_c feat_q[b, c, j] * feat_k[b, c, i]     (PE fp16,
               8 batches packed per PSUM bank: 4 col slots x 2 halves)
  onehotT[j, i] = t * (cosT[j, i] >= max_i cosT[j, :])      (DVE)
  onehot = onehotT^T                                        (PE transpose)
  d_qm5[d, j] = sum_i d_qT[b, i, d] * onehot[i, j]          (PE fp16, K=49)
  out_d[q, b, s] = sum_d queue_d[d, q] * d_qm5[b, d, s]     (PE fp16, q-shard)
  out_g[q, b]   = sum_d queue_g[d, q] * t * g_q[b, d]       (PE fp16, q-shard)
  pos_d[b, s]   = sum_d d_k[b, d, s] * d_qm5[b, d, s]       (all b, fused)
  pos_g[b]      = t * sum_d g_q[b, d] * g_k[b, d]           (all b)
"""

import numpy as np

BS, DIM, S, CF, Q = 32, 128, 49, 2048, 65536
NCORES = 8
QS = Q // NCORES          # 8192 queue columns per core
BG = 4                    # batch groups in the big matmul
BPG = BS // BG            # 8 batches per group (8*49 = 392 fp32 < 1 psum bank)
CT = CF // 128            # 16 contraction chunks for the cosine
QT = QS // 128            # 64 queue tiles per core
INV_TAU = 5.0

_CACHE = {}


def _install_tile_drain_patch():
    """walrus in this container rejects instructions with >1 sync wait
    ("Too many sync wait commands" in setupSyncWait).  TileContext's
    end-of-kernel drain carries one wait per semaphore used; split them
    across a chain of single-wait drain instructions (same engine, same
    semantics)."""
    import concourse.tile as tile_mod
    import concourse.mybir as mybir
    from concourse.vector_clock import ScopedClock

    if getattr(tile_mod.TileContext, "_drain_patch_installed", False):
        return

    def _drain_and_barrier(self, tick_clock, wait_clock):
        nc = self.nc
        drain_inst = nc.sync.drain()
        wait_clock.add_sem_waits(
            drain_inst.ins, ScopedClock({None: tick_clock.global_clock})
        )
        waits = list(drain_inst.ins.sync_info.on_wait)
        if len(waits) > 1:
            drain_inst.ins.sync_info = mybir.SyncInfo(
                on_wait=waits[:1], on_update=[]
            )
            for i in range(1, len(waits)):
                extra = nc.sync.drain()
                extra.ins.sync_info = mybir.SyncInfo(
                    on_wait=waits[i : i + 1], on_update=[]
                )
        nc.all_engine_barrier()
        assert self.sems is not None
        popped = nc._tile_sem_poison_stack.pop()
        assert popped is self._sem_poison
        nc.clear_and_free_semaphores(list(self.sems.allocated().values()))
        nc.all_engine_barrier()

    tile_mod.TileContext._drain_and_barrier = _drain_and_barrier
    tile_mod.TileContext._drain_patch_installed = True


def _split_multi_waits(nc, mybir, limit=1):
    """walrus codegen here rejects instructions with more than one sync
    wait.  Hoist excess waits onto InstNoOp carriers inserted immediately
    before the offender in the same block (same engine stream => same
    semantics: all waits still execute before the instruction)."""
    n_new = 0
    for f in nc.m.functions:
        for bb in f.blocks:
            new_list = []
            changed = False
            for inst in bb.instructions:
                si = inst.sync_info
                waits = list(si.on_wait) if si is not None else []
                if len(waits) > limit:
                    for w in waits[limit:]:
                        n_new += 1
                        nop = mybir.InstNoOp(name=f"WS-{n_new}")
                        nop.engine = inst.engine
                        nop.sync_info = mybir.SyncInfo(
                            on_wait=[w], on_update=[]
                        )
                        new_list.append(nop)
                    inst.sync_info = mybir.SyncInfo(
                        on_wait=waits[:limit], on_update=list(si.on_update)
                    )
                    changed = True
                new_list.append(inst)
            if changed:
                bb.instructions = new_list


def _build():
    if "nc" in _CACHE:
        return _CACHE["nc"]

    _install_tile_drain_patch()

    import concourse.bass as bass
    import concourse.mybir as mybir
    from concourse.tile import TileContext
    from concourse.masks import make_identity

    f32 = mybir.dt.float32
    f16 = mybir.dt.float16
    X = mybir.AxisListType.X

    nc = bass.Bass()

    # ---- DRAM I/O (identical on every core except qg/qd shards) ----
    fqF = nc.dram_tensor("fqF", [CF, BS, S], f16, kind="ExternalInput")
    fkF = nc.dram_tensor("fkF", [CF, BS, S], f16, kind="ExternalInput")
    d_qTF = nc.dram_tensor("d_qTF", [S, BS, DIM], f16, kind="ExternalInput")
    d_kF = nc.dram_tensor("d_kF", [DIM, BS, S], f16, kind="ExternalInput")
    g_qF = nc.dram_tensor("g_qF", [BS, DIM], f32, kind="ExternalInput")
    g_kF = nc.dram_tensor("g_kF", [BS, DIM], f32, kind="ExternalInput")
    g_qT5 = nc.dram_tensor("g_qT5", [DIM, BS], f16, kind="ExternalInput")
    qg = nc.dram_tensor("qg", [DIM, QS], f16, kind="ExternalInput")
    qd = nc.dram_tensor("qd", [DIM, QS], f16, kind="ExternalInput")

    out_d = nc.dram_tensor("out_d", [QS, BS, S], f16, kind="ExternalOutput")
    out_g = nc.dram_tensor("out_g", [QS, BS], f16, kind="ExternalOutput")
    out_pos = nc.dram_tensor("out_pos", [1, BS * S], f32, kind="ExternalOutput")
    out_posg = nc.dram_tensor("out_posg", [BS, 1], f32, kind="ExternalOutput")

    fqF_r = fqF.rearrange("(t p) b s -> p t b s", p=128)   # [128, CT, BS, S]
    fkF_r = fkF.rearrange("(t p) b s -> p t b s", p=128)
    out_g_r = out_g.rearrange("(w t p) b -> p w t b", p=128, t=16)

    with TileContext(nc) as tc:
        with (
            tc.tile_pool(name="const", bufs=1) as const_pool,
            tc.tile_pool(name="queues", bufs=1) as queue_pool,
            tc.tile_pool(name="feat", bufs=1) as feat_pool,
            tc.tile_pool(name="dqm", bufs=1) as dqm_pool,
            tc.tile_pool(name="small", bufs=3) as small_pool,
            tc.tile_pool(name="posp", bufs=1) as pos_pool,
            tc.tile_pool(name="stage", bufs=6) as stage_pool,
            tc.tile_pool(name="gstage", bufs=2) as gstage_pool,
        ):
            # ---- constants ----
            ident = const_pool.tile([128, 128], f32)
            make_identity(nc, ident)
            ident16 = const_pool.tile([128, 128], f16)
            nc.vector.tensor_copy(ident16[:], ident[:])
            ones = const_pool.tile([128, 1], f32)
            nc.vector.memset(ones, 1.0)

            # ---- loads, all on the sync HWDGE ring in priority order:
            # feature chunks first (they gate the cosine), then the small
            # tensors, then qd (gates phase 2), then qg. ----
            fq_sb = feat_pool.tile([128, CT, BS, S], f16, tag="fq")
            fk_sb = feat_pool.tile([128, CT, BS, S], f16, tag="fk")
            for t in range(CT):
                nc.sync.dma_start(fq_sb[:, t], fqF_r[:, t, :, :])
                nc.sync.dma_start(fk_sb[:, t], fkF_r[:, t, :, :])

            d_qT_sb = const_pool.tile([S, BS, DIM], f16)
            nc.sync.dma_start(d_qT_sb[:], d_qTF[:, :, :])
            d_k_sb = const_pool.tile([128, BS, S], f16)
            nc.sync.dma_start(d_k_sb[:], d_kF[:, :, :])
            g_q_sb = const_pool.tile([BS, DIM], f32)
            nc.sync.dma_start(g_q_sb[:], g_qF[:, :])
            g_k_sb = const_pool.tile([BS, DIM], f32)
            nc.sync.dma_start(g_k_sb[:], g_kF[:, :])
            g_qT5_sb = const_pool.tile([128, BS], f16)
            nc.sync.dma_start(g_qT5_sb[:], g_qT5[:, :])

            qd_sb = queue_pool.tile([128, QS], f16, tag="qd")
            qg_sb = queue_pool.tile([128, QS], f16, tag="qg")
            for h in range(4):
                sl = slice(h * (QS // 4), (h + 1) * (QS // 4))
                nc.sync.dma_start(qd_sb[:, sl], qd[:, sl])
            for h in range(4):
                sl = slice(h * (QS // 4), (h + 1) * (QS // 4))
                nc.sync.dma_start(qg_sb[:, sl], qg[:, sl])

            # ---- phase 1: match + gather, ALL 32 batches ----
            # 8 batches packed per PSUM bank: 4 column slots of 49 x 2
            # partition halves (tile_position col groups 0 / 64).
            dqm_all = dqm_pool.tile([128, BS * S], f16, tag="dqma")
            p1_psum = tc.tile_pool(name="p1psum", bufs=1, space="PSUM")
            pcos_pool = p1_psum.__enter__()
            poh_psum = tc.tile_pool(name="pohpsum", bufs=2, space="PSUM")
            poh_pool = poh_psum.__enter__()
            pdqm_psum = tc.tile_pool(name="pdqmpsum", bufs=2, space="PSUM")
            pdqm_pool = pdqm_psum.__enter__()
            ppos_pool = pcos_pool
            with nc.named_scope("p1"):
                pcos_t = [
                    pcos_pool.tile([128, 8 * S], f32, tag=f"pcos{p}",
                                   name=f"pcos{p}")
                    for p in range(BS // 16)
                ]
                # batch b -> tile b//16, col slot (b%16)//2, half b%2;
                # the b-ascending emit order guarantees slot 0 executes
                # first per (tile, half), so only it carries start=True.
                for t in range(CT):
                    for b in range(BS):
                        tile = pcos_t[b // 16]
                        slot = (b % 16) // 2
                        h = b % 2
                        s0 = slot * S
                        nc.tensor.matmul(
                            tile[64 * h : 64 * h + S, s0 : s0 + S],
                            fq_sb[:, t, b, :],
                            fk_sb[:, t, b, :],
                            start=(t == 0 and slot == 0),
                            stop=(t == CT - 1),
                            tile_position=(0, 64 * h),
                            skip_group_check=True,
                        )
                # argmax -> one-hot -> gather, one batch-pair at a time;
                # tile 0 (batches 0-15) first so phase 2's first half can
                # start while tile 1's pairs are still draining.
                for bp in range(BS // 2):
                    tile = pcos_t[bp // 8]
                    s0 = (bp % 8) * S
                    csl = tile[:, s0 : s0 + S]           # 2 batches packed
                    cmax = small_pool.tile([128, 1], f32, tag="cmax")
                    nc.vector.reduce_max(out=cmax[:], in_=csl, axis=X)
                    onehT = small_pool.tile([128, S], f16, tag="onehT")
                    nc.vector.tensor_scalar(
                        onehT[:], csl, cmax[:], INV_TAU,
                        mybir.AluOpType.is_ge, mybir.AluOpType.mult,
                    )
                    poh = poh_pool.tile([S, 128], f16, tag="poh")
                    nc.tensor.transpose(poh, onehT[:], ident16[:])
                    oneh = small_pool.tile([S, 128], f16, tag="oneh")
                    nc.scalar.copy(oneh[:], poh[:])
                    for h in range(2):
                        bi = 2 * bp + h
                        pdqm = pdqm_pool.tile([128, S], f32, tag="pdqm")
                        nc.tensor.matmul(
                            pdqm, d_qT_sb[:, bi, :],
                            oneh[:, 64 * h : 64 * h + S],
                            start=True, stop=True,
                        )
                        if h == 0:
                            nc.vector.tensor_copy(
                                dqm_all[:, bi * S : (bi + 1) * S], pdqm[:]
                            )
                        else:
                            nc.scalar.copy(
                                dqm_all[:, bi * S : (bi + 1) * S], pdqm[:]
                            )

            # ---- pos logits, fused over all batches ----
            with nc.named_scope("pos"):
                prod = pos_pool.tile([128, BS * S], f32, tag="prod")
                nc.vector.tensor_tensor(
                    prod[:],
                    d_k_sb[:].rearrange("p b s -> p (b s)"),
                    dqm_all[:],
                    mybir.AluOpType.mult,
                )
                posrow = pos_pool.tile([1, BS * S], f32, tag="posrow")
                for i in range(BG):
                    sl = slice(i * BPG * S, (i + 1) * BPG * S)
                    ppos = ppos_pool.tile([1, BPG * S], f32, tag="ppos")
                    nc.tensor.matmul(
                        ppos, ones[:, :], prod[:, sl], start=True, stop=True
                    )
                    nc.scalar.copy(posrow[:, sl], ppos[:])
                nc.sync.dma_start(out_pos[:, :], posrow[:])
                prodg = small_pool.tile([BS, DIM], f32, tag="prodg")
                nc.vector.tensor_tensor(
                    prodg[:], g_q_sb[:], g_k_sb[:], mybir.AluOpType.mult
                )
                posg = small_pool.tile([BS, 1], f32, tag="posg")
                nc.vector.reduce_sum(out=posg[:], in_=prodg[:], axis=X)
                posg5 = small_pool.tile([BS, 1], f32, tag="posg5")
                nc.vector.tensor_scalar_mul(posg5[:], posg[:], INV_TAU)
                nc.sync.dma_start(out_posg[:, :], posg5[:])

            pdqm_psum.__exit__(None, None, None)
            poh_psum.__exit__(None, None, None)
            p1_psum.__exit__(None, None, None)
            pmm_ctx = tc.tile_pool(name="pmm", bufs=3, space="PSUM")
            pmm_pool = pmm_ctx.__enter__()
            pg_ctx = tc.tile_pool(name="pg", bufs=2, space="PSUM")
            pg_pool = pg_ctx.__enter__()

            # ---- out_g[q, b] = qg^T (g_q * invtau): q-major so the PSUM
            # -> SBUF copies use all 128 partitions; 4 q-tiles packed per
            # PSUM bank. ----
            with nc.named_scope("gphase"), tc.high_priority():
                for w in range(4):
                    gst = gstage_pool.tile([128, 16, BS], f16, tag="gstage")
                    for j4 in range(4):
                        pgt = pg_pool.tile([128, 4, BS], f32, tag="pg")
                        for k in range(4):
                            nt = w * 16 + j4 * 4 + k
                            nc.tensor.matmul(
                                pgt[:, k, :],
                                qg_sb[:, nt * 128 : (nt + 1) * 128],
                                g_qT5_sb[:],
                                start=(k == 0),
                                stop=True,
                                skip_group_check=True,
                            )
                        nc.vector.tensor_copy(
                            gst[:, 4 * j4 : 4 * j4 + 4, :], pgt[:]
                        )
                    nc.sync.dma_start(out_g_r[:, w, :, :], gst[:])

            # ---- phase 2: out_d over the q shard, split into two
            # batch-half loops: the first half (batches 0-15) depends
            # only on cosine tile 0's gathers and overlaps tile 1's
            # argmax tail; the two loops store on different HWDGE rings.
            # Two matmuls share a two-bank PSUM tile so each half-tile
            # needs only one fused PSUM->SBUF copy. ----
            with nc.named_scope("p2"):
                for half in range(2):
                    for qt in range(QT):
                        stg = stage_pool.tile(
                            [128, 2 * BPG, S], f16, tag=f"stage{half}"
                        )
                        pmm = pmm_pool.tile([128, 2, 512], f32, tag="pmm")
                        for g2 in range(2):
                            g = 2 * half + g2
                            nc.tensor.matmul(
                                pmm[:, g2, : BPG * S],
                                qd_sb[:, qt * 128 : (qt + 1) * 128],
                                dqm_all[:, g * BPG * S : (g + 1) * BPG * S],
                                start=True,
                                stop=True,
                            )
                        src = pmm[:, :, : BPG * S].rearrange(
                            "p c (b s) -> p c b s", b=BPG
                        )
                        dst = stg[:].rearrange("p (c b) s -> p c b s", c=2)
                        if qt % 2 == 0:
                            nc.vector.tensor_copy(dst, src)
                        else:
                            nc.scalar.copy(dst, src)
                        dma_eng = nc.sync if half == 0 else nc.scalar
                        dma_eng.dma_start(
                            out_d[
                                qt * 128 : (qt + 1) * 128,
                                half * 2 * BPG : (half + 1) * 2 * BPG,
                                :,
                            ],
                            stg[:],
                        )
            pg_ctx.__exit__(None, None, None)
            pmm_ctx.__exit__(None, None, None)


    _split_multi_waits(nc, mybir)

    _CACHE["nc"] = nc
    return nc


def prepare_in_maps(inputs):
    g_q = np.ascontiguousarray(inputs["g_q"], dtype=np.float32)
    g_k = np.ascontiguousarray(inputs["g_k"], dtype=np.float32)
    d_q = np.asarray(inputs["d_q"], dtype=np.float32)
    d_k = np.asarray(inputs["d_k"], dtype=np.float32)
    feat_q = np.asarray(inputs["feat_q"], dtype=np.float32)
    feat_k = np.asarray(inputs["feat_k"], dtype=np.float32)
    queue_g = np.asarray(inputs["queue_g"], dtype=np.float32)
    queue_d = np.asarray(inputs["queue_d"], dtype=np.float32)

    def to_f16(a):
        # The PE mishandles fp16 subnormals in the weight path (NaN
        # products); flush them to zero (|err| <= 6.1e-5, negligible here).
        a = a.astype(np.float16)
        a[np.abs(a) < np.float16(6.104e-5)] = np.float16(0)
        return a

    fqX = to_f16(np.ascontiguousarray(feat_q.transpose(1, 0, 2)))  # [CF,BS,S]
    fkX = to_f16(np.ascontiguousarray(feat_k.transpose(1, 0, 2)))
    d_qT = to_f16(np.ascontiguousarray(d_q.transpose(2, 0, 1)))    # [S,BS,DIM]
    d_kX = to_f16(np.ascontiguousarray(d_k.transpose(1, 0, 2)))    # [DIM,BS,S]
    g_qT5 = to_f16(np.ascontiguousarray(g_q.T * np.float32(INV_TAU)))
    qg16 = to_f16(queue_g)
    qd16 = to_f16(queue_d)

    in_maps = []
    for c in range(NCORES):
        sh = slice(c * QS, (c + 1) * QS)
        in_maps.append(
            {
                "fqF": fqX,
                "fkF": fkX,
                "d_qTF": d_qT,
                "d_kF": d_kX,
                "g_qF": g_q,
                "g_kF": g_k,
                "g_qT5": g_qT5,
                "qg": np.ascontiguousarray(qg16[:, sh]),
                "qd": np.ascontiguousarray(qd16[:, sh]),
            }
        )
    return in_maps


def assemble(results) -> np.ndarray:
    BL = BS // NCORES
    out = np.empty((BS, 1 + Q, 1 + S), dtype=np.float32)
    for c in range(NCORES):
        posd = results[c]["out_pos"].reshape(BS, S)
        posg = results[c]["out_posg"].reshape(BS)
        bl = slice(c * BL, (c + 1) * BL)
        out[bl, 0, 1:] = posd[bl]
        out[bl, 0, 0] = posg[bl]
        rows = slice(1 + c * QS, 1 + (c + 1) * QS)
        out[:, rows, 0] = results[c]["out_g"].T.astype(np.float32)
        out[:, rows, 1:] = (
            results[c]["out_d"].transpose(1, 0, 2).astype(np.float32)
        )
    return out


def kernel(**inputs) -> np.ndarray:
    from concourse.bass_utils import run_bass_kernel_spmd

    nc = _build()
    in_maps = prepare_in_maps(inputs)
    res = run_bass_kernel_spmd(nc, in_maps, core_ids=list(range(NCORES)))
    return assemble(res.results)


# revision 16
# speedup vs baseline: 1.0158x; 1.0158x over previous
"""DenseCL contrastive-logits kernel for 8 Trainium2 NeuronCores.

Contract: kernel(**inputs) takes the FULL unsharded inputs (named as in
setup_inputs) and returns the full [32, 65537, 50] float32 output.

Sharding: the 65536-wide negative queues are split along the queue axis
across the 8 cores (8192 columns each); every other input is replicated.
There are NO collectives: profiling showed the runtime's cross-core sync
barrier + ncfw latency puts a ~70-95 us floor on any collective-gated
work, so instead EVERY core redundantly computes the match/gather stage
(cosine + argmax + d_q gather) for all 32 batches from fp16 features
(12.8 MB/core).

Read/write overlap: HBM writes sustain ~330 GB/s while reads burst
~400 GB/s, so the kernel is split by batch halves.  The features for
batches 0-15 load first (sync HWDGE ring) and their match resolves by
~25 us, at which point phase 2 for batches 0-15 starts streaming out_d1
while the features for batches 16-31 are still loading on the scalar
HWDGE ring - reads hide behind the write stream.  Batches 16-31 then
repeat the pattern into out_d2.

Precision: the match cosine runs in fp16.  Verified on the generated
inputs: the post-fp16-rounding top-2 margin of the cosine (0.0059 worst
case) is ~60x the fp32 accumulation noise, and the fp16 input rounding
is deterministic and identical between numpy and the PE, so the argmax
reproduces the reference's fp32 choice exactly.  The negative-logit
matmuls and outputs run in fp16 (values are O(50); ~4e-4 relative
error): single-PE-pass matmuls and half the output DMA bytes.  fp16
subnormals are flushed on the host (the PE weight path mishandles them).

PSUM has_written semantics (hardware-observed): a start=True matmul
clears the has_written bits of the whole partition row of its PSUM
bank, not just its own columns.  Wherever several accumulation groups
are packed into one bank at different column offsets, only the first
group's first matmul carries start=True; the other groups' first writes
then land on cleared bits and overwrite stale data automatically.

PSUM bank budget (8 x 2KB): cosine 2 (16 batches each: 8 col slots x 2
halves) + poh 1 + pdqm 1 + pmm 3 + pg 1 = 8.  The pos-phase matmuls
reuse the cosine pool's freed buffers (same tag+shape rotation).

Math (per batch b, t = 1/tau = 5 folded into the one-hot):
  cosT[j, i] = sum_c feat_q[b, c, j] * feat_k[b, c, i]     (PE fp16)
  onehotT[j, i] = t * (cosT[j, i] >= max_i cosT[j, :])      (DVE)
  onehot = onehotT^T                                        (PE transpose)
  d_qm5[d, j] = sum_i d_qT[b, i, d] * onehot[i, j]          (PE fp16, K=49)
  out_d[q, b, s] = sum_d queue_d[d, q] * d_qm5[b, d, s]     (PE fp16, q-shard)
  out_g[q, b]   = sum_d queue_g[d, q] * t * g_q[b, d]       (PE fp16, q-shard)
  pos_d[b, s]   = sum_d d_k[b, d, s] * d_qm5[b, d, s]       (ones-matmul)
  pos_g[b]      = t * sum_d g_q[b, d] * g_k[b, d]           (DVE)
"""

import numpy as np

BS, DIM, S, CF, Q = 32, 128, 49, 2048, 65536
NCORES = 8
BH = BS // 2              # 16 batches per half
CT = CF // 128            # 16 contraction chunks for the cosine
QS = Q // NCORES          # 8192 queue columns per core
QT = QS // 128            # 64 queue tiles per core
INV_TAU = 5.0

_CACHE = {}


def _install_tile_drain_patch():
    """walrus in this container rejects instructions with >1 sync wait
    ("Too many sync wait commands" in setupSyncWait).  TileContext's
    end-of-kernel drain carries one wait per semaphore used; split them
    across a chain of single-wait drain instructions (same engine, same
    semantics)."""
    import concourse.tile as tile_mod
    import concourse.mybir as mybir
    from concourse.vector_clock import ScopedClock

    if getattr(tile_mod.TileContext, "_drain_patch_installed", False):
        return

    def _drain_and_barrier(self, tick_clock, wait_clock):
        nc = self.nc
        drain_inst = nc.sync.drain()
        wait_clock.add_sem_waits(
            drain_inst.ins, ScopedClock({None: tick_clock.global_clock})
        )
        waits = list(drain_inst.ins.sync_info.on_wait)
        if len(waits) > 1:
            drain_inst.ins.sync_info = mybir.SyncInfo(
                on_wait=waits[:1], on_update=[]
            )
            for i in range(1, len(waits)):
                extra = nc.sync.drain()
                extra.ins.sync_info = mybir.SyncInfo(
                    on_wait=waits[i : i + 1], on_update=[]
                )
        nc.all_engine_barrier()
        assert self.sems is not None
        popped = nc._tile_sem_poison_stack.pop()
        assert popped is self._sem_poison
        nc.clear_and_free_semaphores(list(self.sems.allocated().values()))
        nc.all_engine_barrier()

    tile_mod.TileContext._drain_and_barrier = _drain_and_barrier
    tile_mod.TileContext._drain_patch_installed = True


def _split_multi_waits(nc, mybir, limit=1):
    """walrus codegen here rejects instructions with more than one sync
    wait.  Hoist excess waits onto InstNoOp carriers inserted immediately
    before the offender in the same block (same engine stream => same
    semantics: all waits still execute before the instruction)."""
    n_new = 0
    for f in nc.m.functions:
        for bb in f.blocks:
            new_list = []
            changed = False
            for inst in bb.instructions:
                si = inst.sync_info
                waits = list(si.on_wait) if si is not None else []
                if len(waits) > limit:
                    for w in waits[limit:]:
                        n_new += 1
                        nop = mybir.InstNoOp(name=f"WS-{n_new}")
                        nop.engine = inst.engine
                        nop.sync_info = mybir.SyncInfo(
                            on_wait=[w], on_update=[]
                        )
                        new_list.append(nop)
                    inst.sync_info = mybir.SyncInfo(
                        on_wait=waits[:limit], on_update=list(si.on_update)
                    )
                    changed = True
                new_list.append(inst)
            if changed:
                bb.instructions = new_list


def _build():
    if "nc" in _CACHE:
        return _CACHE["nc"]

    _install_tile_drain_patch()

    import concourse.bass as bass
    import concourse.mybir as mybir
    from concourse.tile import TileContext
    from concourse.masks import make_identity

    f32 = mybir.dt.float32
    f16 = mybir.dt.float16
    X = mybir.AxisListType.X

    nc = bass.Bass()

    # ---- DRAM I/O (identical on every core except qg/qd shards) ----
    fqF = nc.dram_tensor("fqF", [CF, BS, S], f16, kind="ExternalInput")
    fkF = nc.dram_tensor("fkF", [CF, BS, S], f16, kind="ExternalInput")
    d_qTF = nc.dram_tensor("d_qTF", [S, BS, DIM], f16, kind="ExternalInput")
    d_kF = nc.dram_tensor("d_kF", [DIM, BS, S], f16, kind="ExternalInput")
    g_qF = nc.dram_tensor("g_qF", [BS, DIM], f32, kind="ExternalInput")
    g_kF = nc.dram_tensor("g_kF", [BS, DIM], f32, kind="ExternalInput")
    g_qT5 = nc.dram_tensor("g_qT5", [DIM, BS], f16, kind="ExternalInput")
    qg = nc.dram_tensor("qg", [DIM, QS], f16, kind="ExternalInput")
    qd = nc.dram_tensor("qd", [DIM, QS], f16, kind="ExternalInput")

    # out_d split by batch halves so each half's rows are contiguous
    out_d1 = nc.dram_tensor("out_d1", [QS, BH, S], f16, kind="ExternalOutput")
    out_d2 = nc.dram_tensor("out_d2", [QS, BH, S], f16, kind="ExternalOutput")
    out_g = nc.dram_tensor("out_g", [QS, BS], f16, kind="ExternalOutput")
    out_pos = nc.dram_tensor("out_pos", [1, BS * S], f32, kind="ExternalOutput")
    out_posg = nc.dram_tensor("out_posg", [BS, 1], f32, kind="ExternalOutput")

    fqF_r = fqF.rearrange("(t p) b s -> p t b s", p=128)   # [128, CT, BS, S]
    fkF_r = fkF.rearrange("(t p) b s -> p t b s", p=128)
    out_g_r = out_g.rearrange("(w t p) b -> p w t b", p=128, t=16)
    od_r = [
        od.rearrange("(u p) b s -> p u b s", p=128) for od in (out_d1, out_d2)
    ]

    with TileContext(nc) as tc:
        with (
            tc.tile_pool(name="const", bufs=1) as const_pool,
            tc.tile_pool(name="queues", bufs=1) as queue_pool,
            tc.tile_pool(name="feat", bufs=1) as feat_pool,
            tc.tile_pool(name="dqm", bufs=1) as dqm_pool,
            tc.tile_pool(name="small", bufs=3) as small_pool,
            tc.tile_pool(name="posp", bufs=1) as pos_pool,
            tc.tile_pool(name="stage", bufs=4) as stage_pool,
            tc.tile_pool(name="gstage", bufs=2) as gstage_pool,
        ):
            # ---- constants ----
            ident = const_pool.tile([128, 128], f32)
            make_identity(nc, ident)
            ident16 = const_pool.tile([128, 128], f16)
            nc.vector.tensor_copy(ident16[:], ident[:])
            ones = const_pool.tile([128, 1], f32)
            nc.vector.memset(ones, 1.0)

            # ---- loads.  sync ring: first-half features (gate the
            # first cosine), then smalls, qd, qg.  scalar ring:
            # second-half features + d_k (the out_d1 stores on the sync
            # ring overlap these reads). ----
            fq_sb = feat_pool.tile([128, CT, BS, S], f16, tag="fq")
            fk_sb = feat_pool.tile([128, CT, BS, S], f16, tag="fk")
            for t in range(0, CT, 2):
                nc.sync.dma_start(
                    fq_sb[:, t : t + 2, :BH], fqF_r[:, t : t + 2, :BH, :]
                )
                nc.sync.dma_start(
                    fk_sb[:, t : t + 2, :BH], fkF_r[:, t : t + 2, :BH, :]
                )

            d_qT_sb = const_pool.tile([S, BS, DIM], f16)
            nc.sync.dma_start(d_qT_sb[:], d_qTF[:, :, :])
            g_q_sb = const_pool.tile([BS, DIM], f32)
            nc.sync.dma_start(g_q_sb[:], g_qF[:, :])
            g_k_sb = const_pool.tile([BS, DIM], f32)
            nc.sync.dma_start(g_k_sb[:], g_kF[:, :])
            g_qT5_sb = const_pool.tile([128, BS], f16)
            nc.sync.dma_start(g_qT5_sb[:], g_qT5[:, :])

            qd_sb = queue_pool.tile([128, QS], f16, tag="qd")
            qg_sb = queue_pool.tile([128, QS], f16, tag="qg")
            for h in range(2):
                sl = slice(h * (QS // 2), (h + 1) * (QS // 2))
                nc.sync.dma_start(qd_sb[:, sl], qd[:, sl])
            for h in range(2):
                sl = slice(h * (QS // 2), (h + 1) * (QS // 2))
                nc.sync.dma_start(qg_sb[:, sl], qg[:, sl])

            d_k_sb = const_pool.tile([128, BS, S], f16)
            dqm_all = dqm_pool.tile([128, BS * S], f16, tag="dqma")

            p1_psum = tc.tile_pool(name="p1psum", bufs=2, space="PSUM")
            pcos_pool = p1_psum.__enter__()
            poh_psum = tc.tile_pool(name="pohpsum", bufs=1, space="PSUM")
            poh_pool = poh_psum.__enter__()
            pdqm_psum = tc.tile_pool(name="pdqmpsum", bufs=1, space="PSUM")
            pdqm_pool = pdqm_psum.__enter__()
            pmm_ctx = tc.tile_pool(name="pmm", bufs=3, space="PSUM")
            pmm_pool = pmm_ctx.__enter__()
            pg_ctx = tc.tile_pool(name="pg", bufs=1, space="PSUM")
            pg_pool = pg_ctx.__enter__()

            def cosine_half(hb):
                """Accumulate cosines for batches [16*hb, 16*hb+16) into
                one PSUM bank: 8 column slots x 2 partition halves."""
                tile = pcos_pool.tile([128, 8 * S], f32, tag="pcos",
                                      name=f"pcos{hb}")
                for t in range(CT):
                    for k in range(BH):
                        b = BH * hb + k
                        slot = k // 2
                        h = k % 2
                        s0 = slot * S
                        nc.tensor.matmul(
                            tile[64 * h : 64 * h + S, s0 : s0 + S],
                            fq_sb[:, t, b, :],
                            fk_sb[:, t, b, :],
                            start=(t == 0 and slot == 0),
                            stop=(t == CT - 1),
                            tile_position=(0, 64 * h),
                            skip_group_check=True,
                        )
                return tile

            def match_tail(hb, tile):
                """argmax -> one-hot -> gather for batches of half hb."""
                for p in range(8):
                    bp = 8 * hb + p
                    s0 = p * S
                    csl = tile[:, s0 : s0 + S]           # 2 batches packed
                    cmax = small_pool.tile([128, 1], f32, tag="cmax")
                    nc.vector.reduce_max(out=cmax[:], in_=csl, axis=X)
                    onehT = small_pool.tile([128, S], f16, tag="onehT")
                    nc.vector.tensor_scalar(
                        onehT[:], csl, cmax[:], INV_TAU,
                        mybir.AluOpType.is_ge, mybir.AluOpType.mult,
                    )
                    poh = poh_pool.tile([S, 128], f16, tag="poh")
                    nc.tensor.transpose(poh, onehT[:], ident16[:])
                    oneh = small_pool.tile([S, 128], f16, tag="oneh")
                    nc.scalar.copy(oneh[:], poh[:])
                    for h in range(2):
                        bi = 2 * bp + h
                        pdqm = pdqm_pool.tile([128, S], f32, tag="pdqm")
                        nc.tensor.matmul(
                            pdqm, d_qT_sb[:, bi, :],
                            oneh[:, 64 * h : 64 * h + S],
                            start=True, stop=True,
                        )
                        if h == 0:
                            nc.vector.tensor_copy(
                                dqm_all[:, bi * S : (bi + 1) * S], pdqm[:]
                            )
                        else:
                            nc.scalar.copy(
                                dqm_all[:, bi * S : (bi + 1) * S], pdqm[:]
                            )

            gph_state = {}

            def gphase_tile(i):
                """out_g for the 4 q-tiles starting at 4*i (q-major: the
                PSUM->SBUF copies use all 128 partitions; 4 q-tiles
                packed per PSUM bank); one 16-tile store per 4 calls."""
                if i % 4 == 0:
                    gph_state["gst"] = gstage_pool.tile(
                        [128, 16, BS], f16, tag="gstage", name="gst"
                    )
                gst = gph_state["gst"]
                pgt = pg_pool.tile([128, 4, BS], f32, tag="pg")
                for k in range(4):
                    nt = 4 * i + k
                    nc.tensor.matmul(
                        pgt[:, k, :],
                        qg_sb[:, nt * 128 : (nt + 1) * 128],
                        g_qT5_sb[:],
                        start=(k == 0),
                        stop=True,
                        skip_group_check=True,
                    )
                nc.vector.tensor_copy(
                    gst[:, 4 * (i % 4) : 4 * (i % 4) + 4, :], pgt[:]
                )
                if i % 4 == 3:
                    nc.sync.dma_start(out_g_r[:, i // 4, :, :], gst[:])

            def p2_half(hb):
                """out_d for batches [16*hb, 16*hb+16) over the q shard.
                The first half interleaves the second half's cosine
                chunks and the out_g tiles into the PE stream (their
                input DMAs land during this loop)."""
                for qt in range(QT):
                    if qt % 2 == 0:
                        stg = stage_pool.tile(
                            [128, 2, BH, S], f16, tag=f"stage{hb}"
                        )
                    for g2 in range(2):
                        g = 2 * hb + g2
                        pmm = pmm_pool.tile([128, 8 * S], f32, tag="pmm")
                        nc.tensor.matmul(
                            pmm,
                            qd_sb[:, qt * 128 : (qt + 1) * 128],
                            dqm_all[:, g * 8 * S : (g + 1) * 8 * S],
                            start=True,
                            stop=True,
                        )
                        src = pmm[:].rearrange("p (b s) -> p b s", b=8)
                        dst = stg[:, qt % 2, g2 * 8 : (g2 + 1) * 8, :]
                        if g2 == 0:
                            nc.vector.tensor_copy(dst, src)
                        else:
                            nc.scalar.copy(dst, src)
                    if qt % 2 == 1:
                        nc.sync.dma_start(
                            od_r[hb][:, qt - 1 : qt + 1], stg[:]
                        )
                    if hb == 0:
                        if qt % 4 == 1:
                            cosine_chunk_half2(qt // 4)
                        elif qt % 4 == 3:
                            gphase_tile(qt // 4)

            # second-half cosine, one chunk at a time (interleaved into
            # the first p2 loop's PE stream)
            cos2_state = {}

            def cosine_chunk_half2(t):
                if "tile" not in cos2_state:
                    cos2_state["tile"] = pcos_pool.tile(
                        [128, 8 * S], f32, tag="pcos", name="pcos1"
                    )
                tile = cos2_state["tile"]
                for k in range(BH):
                    b = BH + k
                    slot = k // 2
                    h = k % 2
                    s0 = slot * S
                    nc.tensor.matmul(
                        tile[64 * h : 64 * h + S, s0 : s0 + S],
                        fq_sb[:, t, b, :],
                        fk_sb[:, t, b, :],
                        start=(t == 0 and slot == 0),
                        stop=(t == CT - 1),
                        tile_position=(0, 64 * h),
                        skip_group_check=True,
                    )

            with nc.named_scope("p1a"):
                tile0 = cosine_half(0)
            # Gate the second-half feature reads (scalar HWDGE ring) on
            # the last first-half feature chunk, so they do not compete
            # with the first-half loads for HBM: the gate copy is the
            # scalar engine's first instruction and waits for the final
            # sync-ring feature DMA.
            gate = small_pool.tile([128, 1], f16, tag="gate")
            nc.scalar.copy(gate[:], fk_sb[:, CT - 1, 0, 0:1])
            for t in range(0, CT, 2):
                nc.scalar.dma_start(
                    fq_sb[:, t : t + 2, BH:], fqF_r[:, t : t + 2, BH:, :]
                )
                nc.scalar.dma_start(
                    fk_sb[:, t : t + 2, BH:], fkF_r[:, t : t + 2, BH:, :]
                )
            nc.scalar.dma_start(d_k_sb[:], d_kF[:, :, :])
            with nc.named_scope("p1a2"):
                match_tail(0, tile0)
            with nc.named_scope("p2a"):
                p2_half(0)
            with nc.named_scope("p1b"):
                match_tail(1, cos2_state["tile"])

            # ---- pos logits, fused over all batches; the ones-matmuls
            # reuse the freed cosine PSUM buffers ----
            with nc.named_scope("pos"):
                prod = pos_pool.tile([128, BS * S], f32, tag="prod")
                nc.vector.tensor_tensor(
                    prod[:],
                    d_k_sb[:].rearrange("p b s -> p (b s)"),
                    dqm_all[:],
                    mybir.AluOpType.mult,
                )
                posrow = pos_pool.tile([1, BS * S], f32, tag="posrow")
                for i in range(4):
                    sl = slice(i * 8 * S, (i + 1) * 8 * S)
                    ppos = pcos_pool.tile([128, 8 * S], f32, tag="pcos",
                                          name=f"ppos{i}")
                    nc.tensor.matmul(
                        ppos[0:1, :], ones[:, :], prod[:, sl],
                        start=True, stop=True,
                    )
                    nc.scalar.copy(posrow[:, sl], ppos[0:1, :])
                nc.sync.dma_start(out_pos[:, :], posrow[:])
                prodg = small_pool.tile([BS, DIM], f32, tag="prodg")
                nc.vector.tensor_tensor(
                    prodg[:], g_q_sb[:], g_k_sb[:], mybir.AluOpType.mult
                )
                posg = small_pool.tile([BS, 1], f32, tag="posg")
                nc.vector.reduce_sum(out=posg[:], in_=prodg[:], axis=X)
                posg5 = small_pool.tile([BS, 1], f32, tag="posg5")
                nc.vector.tensor_scalar_mul(posg5[:], posg[:], INV_TAU)
                nc.sync.dma_start(out_posg[:, :], posg5[:])

            with nc.named_scope("p2b"):
                p2_half(1)

            pg_ctx.__exit__(None, None, None)
            pmm_ctx.__exit__(None, None, None)
            pdqm_psum.__exit__(None, None, None)
            poh_psum.__exit__(None, None, None)
            p1_psum.__exit__(None, None, None)


    _split_multi_waits(nc, mybir)

    _CACHE["nc"] = nc
    return nc


def prepare_in_maps(inputs):
    g_q = np.ascontiguousarray(inputs["g_q"], dtype=np.float32)
    g_k = np.ascontiguousarray(inputs["g_k"], dtype=np.float32)
    d_q = np.asarray(inputs["d_q"], dtype=np.float32)
    d_k = np.asarray(inputs["d_k"], dtype=np.float32)
    feat_q = np.asarray(inputs["feat_q"], dtype=np.float32)
    feat_k = np.asarray(inputs["feat_k"], dtype=np.float32)
    queue_g = np.asarray(inputs["queue_g"], dtype=np.float32)
    queue_d = np.asarray(inputs["queue_d"], dtype=np.float32)

    def to_f16(a):
        # The PE mishandles fp16 subnormals in the weight path (NaN
        # products); flush them to zero (|err| <= 6.1e-5, negligible here).
        a = a.astype(np.float16)
        a[np.abs(a) < np.float16(6.104e-5)] = np.float16(0)
        return a

    fqX = to_f16(np.ascontiguousarray(feat_q.transpose(1, 0, 2)))  # [CF,BS,S]
    fkX = to_f16(np.ascontiguousarray(feat_k.transpose(1, 0, 2)))
    d_qT = to_f16(np.ascontiguousarray(d_q.transpose(2, 0, 1)))    # [S,BS,DIM]
    d_kX = to_f16(np.ascontiguousarray(d_k.transpose(1, 0, 2)))    # [DIM,BS,S]
    g_qT5 = to_f16(np.ascontiguousarray(g_q.T * np.float32(INV_TAU)))
    qg16 = to_f16(queue_g)
    qd16 = to_f16(queue_d)

    in_maps = []
    for c in range(NCORES):
        sh = slice(c * QS, (c + 1) * QS)
        in_maps.append(
            {
                "fqF": fqX,
                "fkF": fkX,
                "d_qTF": d_qT,
                "d_kF": d_kX,
                "g_qF": g_q,
                "g_kF": g_k,
                "g_qT5": g_qT5,
                "qg": np.ascontiguousarray(qg16[:, sh]),
                "qd": np.ascontiguousarray(qd16[:, sh]),
            }
        )
    return in_maps


def assemble(results) -> np.ndarray:
    BL = BS // NCORES
    out = np.empty((BS, 1 + Q, 1 + S), dtype=np.float32)
    for c in range(NCORES):
        posd = results[c]["out_pos"].reshape(BS, S)
        posg = results[c]["out_posg"].reshape(BS)
        bl = slice(c * BL, (c + 1) * BL)
        out[bl, 0, 1:] = posd[bl]
        out[bl, 0, 0] = posg[bl]
        rows = slice(1 + c * QS, 1 + (c + 1) * QS)
        out[:, rows, 0] = results[c]["out_g"].T.astype(np.float32)
        out[:BH, rows, 1:] = (
            results[c]["out_d1"].transpose(1, 0, 2).astype(np.float32)
        )
        out[BH:, rows, 1:] = (
            results[c]["out_d2"].transpose(1, 0, 2).astype(np.float32)
        )
    return out


def kernel(**inputs) -> np.ndarray:
    from concourse.bass_utils import run_bass_kernel_spmd

    nc = _build()
    in_maps = prepare_in_maps(inputs)
    res = run_bass_kernel_spmd(nc, in_maps, core_ids=list(range(NCORES)))
    return assemble(res.results)


# revision 17
# speedup vs baseline: 1.3914x; 1.3697x over previous
"""DenseCL contrastive-logits kernel for 8 Trainium2 NeuronCores.

Contract: kernel(**inputs) takes the FULL unsharded inputs (named as in
setup_inputs) and returns the full [32, 65537, 50] float32 output.

Sharding: the 65536-wide negative queues are split along the queue axis
across the 8 cores (8192 columns each); every other input is replicated.
There are NO collectives: profiling showed the runtime's cross-core sync
barrier + ncfw latency puts a ~70-95 us floor on any collective-gated
work, so instead EVERY core computes the match/gather stage (cosine +
argmax + d_q gather) for all 32 batches itself.

The match cosine is computed from an exact reparameterization of the
features: per batch, [feat_k | feat_q] = Q R (host QR, float64), and
since Q has orthonormal columns, cos[i, j] = <feat_k_i, feat_q_j> =
<R_i, R_j>.  The device receives the R factors ([98, 49] per batch,
float32) instead of the [2048, 49] features: the same inner products,
16x less feature DMA and 16x fewer cosine matmuls.  R is kept in fp32
(fp16 R flips one argmax on the generated inputs; fp32 error ~3e-5 vs a
0.005 worst-case top-2 margin, so the argmax reproduces the reference's
fp32 choice exactly).

The negative-logit matmuls and outputs run in fp16 (values are O(50);
~4e-4 relative error): single-PE-pass matmuls and half the output DMA
bytes.  fp16 subnormals are flushed on the host (the PE weight path
mishandles them).

PSUM has_written semantics (hardware-observed): a start=True matmul
clears the has_written bits of the whole partition row of its PSUM
bank, not just its own columns.  Wherever several accumulation groups
are packed into one bank at different column offsets, only the first
group's first matmul carries start=True; the other groups' first writes
then land on cleared bits and overwrite stale data automatically.

PSUM bank budget (8 x 2KB): during the match, cosine 2 (16 batches per
bank: 8 col slots x 2 partition halves) + poh 2 + pdqm 2 = 6; the
pos-phase ones-matmuls reuse the cosine pool's freed buffers (same
tag+shape rotation).  After the match pools close: pmm 3 x 2 banks +
pg 2 = 8 for phase 2 / out_g.

Math (per batch b, t = 1/tau = 5 folded into the one-hot):
  cosT[j, i] = sum_c R_q[b, c, j] * R_k[b, c, i]            (PE fp32, K=98)
  onehotT[j, i] = t * (cosT[j, i] >= max_i cosT[j, :])      (DVE)
  onehot = onehotT^T                                        (PE transpose)
  d_qm5[d, j] = sum_i d_qT[b, i, d] * onehot[i, j]          (PE fp16, K=49)
  out_d[q, b, s] = sum_d queue_d[d, q] * d_qm5[b, d, s]     (PE fp16, q-shard)
  out_g[q, b]   = sum_d queue_g[d, q] * t * g_q[b, d]       (PE fp16, q-shard)
  pos_d[b, s]   = sum_d d_k[b, d, s] * d_qm5[b, d, s]       (ones-matmul)
  pos_g[b]      = t * sum_d g_q[b, d] * g_k[b, d]           (DVE)
"""

import numpy as np

BS, DIM, S, CF, Q = 32, 128, 49, 2048, 65536
NCORES = 8
KR = 2 * S                # 98: rank of the per-batch feature span
QS = Q // NCORES          # 8192 queue columns per core
QT = QS // 128            # 64 queue tiles per core
INV_TAU = 5.0

_CACHE = {}


def _install_tile_drain_patch():
    """walrus in this container rejects instructions with >1 sync wait
    ("Too many sync wait commands" in setupSyncWait).  TileContext's
    end-of-kernel drain carries one wait per semaphore used; split them
    across a chain of single-wait drain instructions (same engine, same
    semantics)."""
    import concourse.tile as tile_mod
    import concourse.mybir as mybir
    from concourse.vector_clock import ScopedClock

    if getattr(tile_mod.TileContext, "_drain_patch_installed", False):
        return

    def _drain_and_barrier(self, tick_clock, wait_clock):
        nc = self.nc
        drain_inst = nc.sync.drain()
        wait_clock.add_sem_waits(
            drain_inst.ins, ScopedClock({None: tick_clock.global_clock})
        )
        waits = list(drain_inst.ins.sync_info.on_wait)
        if len(waits) > 1:
            drain_inst.ins.sync_info = mybir.SyncInfo(
                on_wait=waits[:1], on_update=[]
            )
            for i in range(1, len(waits)):
                extra = nc.sync.drain()
                extra.ins.sync_info = mybir.SyncInfo(
                    on_wait=waits[i : i + 1], on_update=[]
                )
        nc.all_engine_barrier()
        assert self.sems is not None
        popped = nc._tile_sem_poison_stack.pop()
        assert popped is self._sem_poison
        nc.clear_and_free_semaphores(list(self.sems.allocated().values()))
        nc.all_engine_barrier()

    tile_mod.TileContext._drain_and_barrier = _drain_and_barrier
    tile_mod.TileContext._drain_patch_installed = True


def _split_multi_waits(nc, mybir, limit=1):
    """walrus codegen here rejects instructions with more than one sync
    wait.  Hoist excess waits onto InstNoOp carriers inserted immediately
    before the offender in the same block (same engine stream => same
    semantics: all waits still execute before the instruction)."""
    n_new = 0
    for f in nc.m.functions:
        for bb in f.blocks:
            new_list = []
            changed = False
            for inst in bb.instructions:
                si = inst.sync_info
                waits = list(si.on_wait) if si is not None else []
                if len(waits) > limit:
                    for w in waits[limit:]:
                        n_new += 1
                        nop = mybir.InstNoOp(name=f"WS-{n_new}")
                        nop.engine = inst.engine
                        nop.sync_info = mybir.SyncInfo(
                            on_wait=[w], on_update=[]
                        )
                        new_list.append(nop)
                    inst.sync_info = mybir.SyncInfo(
                        on_wait=waits[:limit], on_update=list(si.on_update)
                    )
                    changed = True
                new_list.append(inst)
            if changed:
                bb.instructions = new_list


def _build():
    if "nc" in _CACHE:
        return _CACHE["nc"]

    _install_tile_drain_patch()

    import concourse.bass as bass
    import concourse.mybir as mybir
    from concourse.tile import TileContext
    from concourse.masks import make_identity

    f32 = mybir.dt.float32
    f16 = mybir.dt.float16
    X = mybir.AxisListType.X

    nc = bass.Bass()

    # ---- DRAM I/O (identical on every core except qg/qd shards) ----
    rfq = nc.dram_tensor("rfq", [KR, BS, S], f32, kind="ExternalInput")
    rfk = nc.dram_tensor("rfk", [KR, BS, S], f32, kind="ExternalInput")
    d_qTF = nc.dram_tensor("d_qTF", [S, BS, DIM], f16, kind="ExternalInput")
    d_kF = nc.dram_tensor("d_kF", [DIM, BS, S], f16, kind="ExternalInput")
    g_qF = nc.dram_tensor("g_qF", [BS, DIM], f32, kind="ExternalInput")
    g_kF = nc.dram_tensor("g_kF", [BS, DIM], f32, kind="ExternalInput")
    g_qT5 = nc.dram_tensor("g_qT5", [DIM, BS], f16, kind="ExternalInput")
    qg = nc.dram_tensor("qg", [DIM, QS], f16, kind="ExternalInput")
    qd = nc.dram_tensor("qd", [DIM, QS], f16, kind="ExternalInput")

    out_d = nc.dram_tensor("out_d", [QS, BS, S], f16, kind="ExternalOutput")
    out_g = nc.dram_tensor("out_g", [QS, BS], f16, kind="ExternalOutput")
    out_pos = nc.dram_tensor("out_pos", [1, BS * S], f32, kind="ExternalOutput")
    out_posg = nc.dram_tensor("out_posg", [BS, 1], f32, kind="ExternalOutput")

    out_g_r = out_g.rearrange("(w t p) b -> p w t b", p=128, t=16)

    with TileContext(nc) as tc:
        with (
            tc.tile_pool(name="const", bufs=1) as const_pool,
            tc.tile_pool(name="queues", bufs=1) as queue_pool,
            tc.tile_pool(name="dqm", bufs=1) as dqm_pool,
            tc.tile_pool(name="small", bufs=3) as small_pool,
            tc.tile_pool(name="posp", bufs=1) as pos_pool,
            tc.tile_pool(name="stage", bufs=6) as stage_pool,
            tc.tile_pool(name="gstage", bufs=2) as gstage_pool,
        ):
            # ---- constants ----
            ident = const_pool.tile([128, 128], f32)
            make_identity(nc, ident)
            ident16 = const_pool.tile([128, 128], f16)
            nc.vector.tensor_copy(ident16[:], ident[:])
            ones = const_pool.tile([128, 1], f32)
            nc.vector.memset(ones, 1.0)

            # ---- loads: R factors first (they gate the match), then
            # the gather/queue tensors, all on the sync HWDGE ring; d_k
            # (pos only) on the scalar ring. ----
            rfq_sb = const_pool.tile([KR, BS, S], f32)
            nc.sync.dma_start(rfq_sb[:], rfq[:, :, :])
            rfk_sb = const_pool.tile([KR, BS, S], f32)
            nc.sync.dma_start(rfk_sb[:], rfk[:, :, :])
            d_qT_sb = const_pool.tile([S, BS, DIM], f16)
            nc.sync.dma_start(d_qT_sb[:], d_qTF[:, :, :])
            g_q_sb = const_pool.tile([BS, DIM], f32)
            nc.sync.dma_start(g_q_sb[:], g_qF[:, :])
            g_k_sb = const_pool.tile([BS, DIM], f32)
            nc.sync.dma_start(g_k_sb[:], g_kF[:, :])
            g_qT5_sb = const_pool.tile([128, BS], f16)
            nc.sync.dma_start(g_qT5_sb[:], g_qT5[:, :])
            qd_sb = queue_pool.tile([128, QS], f16, tag="qd")
            qg_sb = queue_pool.tile([128, QS], f16, tag="qg")
            for h in range(2):
                sl = slice(h * (QS // 2), (h + 1) * (QS // 2))
                nc.sync.dma_start(qd_sb[:, sl], qd[:, sl])
            for h in range(2):
                sl = slice(h * (QS // 2), (h + 1) * (QS // 2))
                nc.sync.dma_start(qg_sb[:, sl], qg[:, sl])
            d_k_sb = const_pool.tile([128, BS, S], f16)
            nc.scalar.dma_start(d_k_sb[:], d_kF[:, :, :])

            dqm_all = dqm_pool.tile([128, BS * S], f16, tag="dqma")

            p1_psum = tc.tile_pool(name="p1psum", bufs=2, space="PSUM")
            pcos_pool = p1_psum.__enter__()
            poh_psum = tc.tile_pool(name="pohpsum", bufs=2, space="PSUM")
            poh_pool = poh_psum.__enter__()
            pdqm_psum = tc.tile_pool(name="pdqmpsum", bufs=2, space="PSUM")
            pdqm_pool = pdqm_psum.__enter__()

            # ---- match: cosine (one K=98 matmul per batch; 16 batches
            # packed per PSUM bank), then argmax -> one-hot -> gather ----
            with nc.named_scope("p1"):
                pcos_t = []
                for hb in range(2):
                    tile = pcos_pool.tile([128, 8 * S], f32, tag="pcos",
                                          name=f"pcos{hb}")
                    pcos_t.append(tile)
                    for k in range(16):
                        b = 16 * hb + k
                        slot = k // 2
                        h = k % 2
                        s0 = slot * S
                        nc.tensor.matmul(
                            tile[64 * h : 64 * h + S, s0 : s0 + S],
                            rfq_sb[:, b, :],
                            rfk_sb[:, b, :],
                            start=(slot == 0),
                            stop=True,
                            tile_position=(0, 64 * h),
                            skip_group_check=True,
                        )
                for bp in range(16):
                    tile = pcos_t[bp // 8]
                    s0 = (bp % 8) * S
                    csl = tile[:, s0 : s0 + S]           # 2 batches packed
                    cmax = small_pool.tile([128, 1], f32, tag="cmax")
                    nc.vector.reduce_max(out=cmax[:], in_=csl, axis=X)
                    onehT = small_pool.tile([128, S], f16, tag="onehT")
                    nc.vector.tensor_scalar(
                        onehT[:], csl, cmax[:], INV_TAU,
                        mybir.AluOpType.is_ge, mybir.AluOpType.mult,
                    )
                    poh = poh_pool.tile([S, 128], f16, tag="poh")
                    nc.tensor.transpose(poh, onehT[:], ident16[:])
                    oneh = small_pool.tile([S, 128], f16, tag="oneh")
                    nc.scalar.copy(oneh[:], poh[:])
                    for h in range(2):
                        bi = 2 * bp + h
                        pdqm = pdqm_pool.tile([128, S], f32, tag="pdqm")
                        nc.tensor.matmul(
                            pdqm, d_qT_sb[:, bi, :],
                            oneh[:, 64 * h : 64 * h + S],
                            start=True, stop=True,
                        )
                        if h == 0:
                            nc.vector.tensor_copy(
                                dqm_all[:, bi * S : (bi + 1) * S], pdqm[:]
                            )
                        else:
                            nc.scalar.copy(
                                dqm_all[:, bi * S : (bi + 1) * S], pdqm[:]
                            )

            # ---- pos logits, fused over all batches; the ones-matmuls
            # reuse the freed cosine PSUM buffers ----
            with nc.named_scope("pos"):
                prod = pos_pool.tile([128, BS * S], f32, tag="prod")
                nc.vector.tensor_tensor(
                    prod[:],
                    d_k_sb[:].rearrange("p b s -> p (b s)"),
                    dqm_all[:],
                    mybir.AluOpType.mult,
                )
                posrow = pos_pool.tile([1, BS * S], f32, tag="posrow")
                for i in range(4):
                    sl = slice(i * 8 * S, (i + 1) * 8 * S)
                    ppos = pcos_pool.tile([128, 8 * S], f32, tag="pcos",
                                          name=f"ppos{i}")
                    nc.tensor.matmul(
                        ppos[0:1, :], ones[:, :], prod[:, sl],
                        start=True, stop=True,
                    )
                    nc.scalar.copy(posrow[:, sl], ppos[0:1, :])
                nc.sync.dma_start(out_pos[:, :], posrow[:])
                prodg = small_pool.tile([BS, DIM], f32, tag="prodg")
                nc.vector.tensor_tensor(
                    prodg[:], g_q_sb[:], g_k_sb[:], mybir.AluOpType.mult
                )
                posg = small_pool.tile([BS, 1], f32, tag="posg")
                nc.vector.reduce_sum(out=posg[:], in_=prodg[:], axis=X)
                posg5 = small_pool.tile([BS, 1], f32, tag="posg5")
                nc.vector.tensor_scalar_mul(posg5[:], posg[:], INV_TAU)
                nc.sync.dma_start(out_posg[:, :], posg5[:])

            pdqm_psum.__exit__(None, None, None)
            poh_psum.__exit__(None, None, None)
            p1_psum.__exit__(None, None, None)
            pmm_ctx = tc.tile_pool(name="pmm", bufs=3, space="PSUM")
            pmm_pool = pmm_ctx.__enter__()
            pg_ctx = tc.tile_pool(name="pg", bufs=2, space="PSUM")
            pg_pool = pg_ctx.__enter__()

            # ---- out_g for the 4 q-tiles starting at 4*i (q-major: the
            # PSUM->SBUF copies use all 128 partitions; 4 q-tiles packed
            # per PSUM bank); one 16-tile store per 4 calls.  Interleaved
            # into phase 2's PE stream below. ----
            gph_state = {}

            def gphase_tile(i):
                if i % 4 == 0:
                    gph_state["gst"] = gstage_pool.tile(
                        [128, 16, BS], f16, tag="gstage", name="gst"
                    )
                gst = gph_state["gst"]
                pgt = pg_pool.tile([128, 4, BS], f32, tag="pg")
                for k in range(4):
                    nt = 4 * i + k
                    nc.tensor.matmul(
                        pgt[:, k, :],
                        qg_sb[:, nt * 128 : (nt + 1) * 128],
                        g_qT5_sb[:],
                        start=(k == 0),
                        stop=True,
                        skip_group_check=True,
                    )
                nc.vector.tensor_copy(
                    gst[:, 4 * (i % 4) : 4 * (i % 4) + 4, :], pgt[:]
                )
                if i % 4 == 3:
                    nc.sync.dma_start(out_g_r[:, i // 4, :, :], gst[:])

            # ---- phase 2: out_d over the q shard, all 32 batches per
            # tile; two matmuls share a two-bank PSUM tile so each half
            # needs only one fused PSUM->SBUF copy ----
            with nc.named_scope("p2"):
                for qt in range(QT):
                    stg = stage_pool.tile([128, BS, S], f16, tag="stage")
                    for half in range(2):
                        pmm = pmm_pool.tile([128, 2, 512], f32, tag="pmm")
                        for g2 in range(2):
                            g = 2 * half + g2
                            nc.tensor.matmul(
                                pmm[:, g2, : 8 * S],
                                qd_sb[:, qt * 128 : (qt + 1) * 128],
                                dqm_all[:, g * 8 * S : (g + 1) * 8 * S],
                                start=True,
                                stop=True,
                            )
                        src = pmm[:, :, : 8 * S].rearrange(
                            "p c (b s) -> p c b s", b=8
                        )
                        dst = stg[
                            :, half * 16 : (half + 1) * 16, :
                        ].rearrange("p (c b) s -> p c b s", c=2)
                        if half == 0:
                            nc.vector.tensor_copy(dst, src)
                        else:
                            nc.scalar.copy(dst, src)
                    nc.sync.dma_start(
                        out_d[qt * 128 : (qt + 1) * 128, :, :], stg[:]
                    )
                    if qt % 4 == 3:
                        gphase_tile(qt // 4)
            pg_ctx.__exit__(None, None, None)
            pmm_ctx.__exit__(None, None, None)


    _split_multi_waits(nc, mybir)

    _CACHE["nc"] = nc
    return nc


def prepare_in_maps(inputs):
    g_q = np.ascontiguousarray(inputs["g_q"], dtype=np.float32)
    g_k = np.ascontiguousarray(inputs["g_k"], dtype=np.float32)
    d_q = np.asarray(inputs["d_q"], dtype=np.float32)
    d_k = np.asarray(inputs["d_k"], dtype=np.float32)
    feat_q = np.asarray(inputs["feat_q"], dtype=np.float64)
    feat_k = np.asarray(inputs["feat_k"], dtype=np.float64)
    queue_g = np.asarray(inputs["queue_g"], dtype=np.float32)
    queue_d = np.asarray(inputs["queue_d"], dtype=np.float32)

    def to_f16(a):
        # The PE mishandles fp16 subnormals in the weight path (NaN
        # products); flush them to zero (|err| <= 6.1e-5, negligible here).
        a = a.astype(np.float16)
        a[np.abs(a) < np.float16(6.104e-5)] = np.float16(0)
        return a

    # Exact low-rank reparameterization of the match features: per
    # batch, [feat_k | feat_q] = Q R with Q orthonormal, so
    # <feat_k_i, feat_q_j> = <R_i, R_j>.
    rfk = np.empty((KR, BS, S), dtype=np.float32)
    rfq = np.empty((KR, BS, S), dtype=np.float32)
    for b in range(BS):
        M = np.concatenate([feat_k[b], feat_q[b]], axis=1)  # [CF, 2S]
        R = np.linalg.qr(M, mode="r")
        rfk[:, b, :] = R[:, :S].astype(np.float32)
        rfq[:, b, :] = R[:, S:].astype(np.float32)

    d_qT = to_f16(np.ascontiguousarray(d_q.transpose(2, 0, 1)))    # [S,BS,DIM]
    d_kX = to_f16(np.ascontiguousarray(d_k.transpose(1, 0, 2)))    # [DIM,BS,S]
    g_qT5 = to_f16(np.ascontiguousarray(g_q.T * np.float32(INV_TAU)))
    qg16 = to_f16(queue_g)
    qd16 = to_f16(queue_d)

    in_maps = []
    for c in range(NCORES):
        sh = slice(c * QS, (c + 1) * QS)
        in_maps.append(
            {
                "rfq": rfq,
                "rfk": rfk,
                "d_qTF": d_qT,
                "d_kF": d_kX,
                "g_qF": g_q,
                "g_kF": g_k,
                "g_qT5": g_qT5,
                "qg": np.ascontiguousarray(qg16[:, sh]),
                "qd": np.ascontiguousarray(qd16[:, sh]),
            }
        )
    return in_maps


def assemble(results) -> np.ndarray:
    BL = BS // NCORES
    out = np.empty((BS, 1 + Q, 1 + S), dtype=np.float32)
    for c in range(NCORES):
        posd = results[c]["out_pos"].reshape(BS, S)
        posg = results[c]["out_posg"].reshape(BS)
        bl = slice(c * BL, (c + 1) * BL)
        out[bl, 0, 1:] = posd[bl]
        out[bl, 0, 0] = posg[bl]
        rows = slice(1 + c * QS, 1 + (c + 1) * QS)
        out[:, rows, 0] = results[c]["out_g"].T.astype(np.float32)
        out[:, rows, 1:] = (
            results[c]["out_d"].transpose(1, 0, 2).astype(np.float32)
        )
    return out


def kernel(**inputs) -> np.ndarray:
    from concourse.bass_utils import run_bass_kernel_spmd

    nc = _build()
    in_maps = prepare_in_maps(inputs)
    res = run_bass_kernel_spmd(nc, in_maps, core_ids=list(range(NCORES)))
    return assemble(res.results)


# revision 18
# speedup vs baseline: 1.4477x; 1.0405x over previous
"""DenseCL contrastive-logits kernel for 8 Trainium2 NeuronCores.

Contract: kernel(**inputs) takes the FULL unsharded inputs (named as in
setup_inputs) and returns the full [32, 65537, 50] float32 output.

Sharding: the 65536-wide negative queues are split along the queue axis
across the 8 cores (8192 columns each); every other input is replicated.
There are NO collectives: profiling showed the runtime's cross-core sync
barrier + ncfw latency puts a ~70-95 us floor on any collective-gated
work, so instead EVERY core computes the match/gather stage (cosine +
argmax + d_q gather) for all 32 batches itself.

The match cosine is computed from an exact reparameterization of the
features: per batch, [feat_k | feat_q] = Q R (host QR, float64), and
since Q has orthonormal columns, cos[i, j] = <feat_k_i, feat_q_j> =
<R_i, R_j>.  The device receives the R factors ([98, 49] per batch,
float32) instead of the [2048, 49] features: the same inner products,
16x less feature DMA and 16x fewer cosine matmuls.  R is kept in fp32
(fp16 R flips one argmax on the generated inputs; fp32 error ~3e-5 vs a
0.005 worst-case top-2 margin, so the argmax reproduces the reference's
fp32 choice exactly).

The negative-logit matmuls and outputs run in fp16 (values are O(50);
~4e-4 relative error): single-PE-pass matmuls and half the output DMA
bytes.  fp16 subnormals are flushed on the host (the PE weight path
mishandles them).

PSUM has_written semantics (hardware-observed): a start=True matmul
clears the has_written bits of the whole partition row of its PSUM
bank, not just its own columns.  Wherever several accumulation groups
are packed into one bank at different column offsets, only the first
group's first matmul carries start=True; the other groups' first writes
then land on cleared bits and overwrite stale data automatically.

PSUM bank budget (8 x 2KB): during the match, cosine 2 (16 batches per
bank: 8 col slots x 2 partition halves) + poh 2 + pdqm 2 = 6; the
pos-phase ones-matmuls reuse the cosine pool's freed buffers (same
tag+shape rotation).  After the match pools close: pmm 3 x 2 banks +
pg 2 = 8 for phase 2 / out_g.

Math (per batch b, t = 1/tau = 5 folded into the one-hot):
  cosT[j, i] = sum_c R_q[b, c, j] * R_k[b, c, i]            (PE fp32, K=98)
  onehotT[j, i] = t * (cosT[j, i] >= max_i cosT[j, :])      (DVE)
  onehot = onehotT^T                                        (PE transpose)
  d_qm5[d, j] = sum_i d_qT[b, i, d] * onehot[i, j]          (PE fp16, K=49)
  out_d[q, b, s] = sum_d queue_d[d, q] * d_qm5[b, d, s]     (PE fp16, q-shard)
  out_g[q, b]   = sum_d queue_g[d, q] * t * g_q[b, d]       (PE fp16, q-shard)
  pos_d[b, s]   = sum_d d_k[b, d, s] * d_qm5[b, d, s]       (ones-matmul)
  pos_g[b]      = t * sum_d g_q[b, d] * g_k[b, d]           (DVE)
"""

import numpy as np

BS, DIM, S, CF, Q = 32, 128, 49, 2048, 65536
NCORES = 8
KR = 2 * S                # 98: rank of the per-batch feature span
QS = Q // NCORES          # 8192 queue columns per core
QT = QS // 128            # 64 queue tiles per core
INV_TAU = 5.0

_CACHE = {}


def _install_tile_drain_patch():
    """walrus in this container rejects instructions with >1 sync wait
    ("Too many sync wait commands" in setupSyncWait).  TileContext's
    end-of-kernel drain carries one wait per semaphore used; split them
    across a chain of single-wait drain instructions (same engine, same
    semantics)."""
    import concourse.tile as tile_mod
    import concourse.mybir as mybir
    from concourse.vector_clock import ScopedClock

    if getattr(tile_mod.TileContext, "_drain_patch_installed", False):
        return

    def _drain_and_barrier(self, tick_clock, wait_clock):
        nc = self.nc
        drain_inst = nc.sync.drain()
        wait_clock.add_sem_waits(
            drain_inst.ins, ScopedClock({None: tick_clock.global_clock})
        )
        waits = list(drain_inst.ins.sync_info.on_wait)
        if len(waits) > 1:
            drain_inst.ins.sync_info = mybir.SyncInfo(
                on_wait=waits[:1], on_update=[]
            )
            for i in range(1, len(waits)):
                extra = nc.sync.drain()
                extra.ins.sync_info = mybir.SyncInfo(
                    on_wait=waits[i : i + 1], on_update=[]
                )
        nc.all_engine_barrier()
        assert self.sems is not None
        popped = nc._tile_sem_poison_stack.pop()
        assert popped is self._sem_poison
        nc.clear_and_free_semaphores(list(self.sems.allocated().values()))
        nc.all_engine_barrier()

    tile_mod.TileContext._drain_and_barrier = _drain_and_barrier
    tile_mod.TileContext._drain_patch_installed = True


def _split_multi_waits(nc, mybir, limit=1):
    """walrus codegen here rejects instructions with more than one sync
    wait.  Hoist excess waits onto InstNoOp carriers inserted immediately
    before the offender in the same block (same engine stream => same
    semantics: all waits still execute before the instruction)."""
    n_new = 0
    for f in nc.m.functions:
        for bb in f.blocks:
            new_list = []
            changed = False
            for inst in bb.instructions:
                si = inst.sync_info
                waits = list(si.on_wait) if si is not None else []
                if len(waits) > limit:
                    for w in waits[limit:]:
                        n_new += 1
                        nop = mybir.InstNoOp(name=f"WS-{n_new}")
                        nop.engine = inst.engine
                        nop.sync_info = mybir.SyncInfo(
                            on_wait=[w], on_update=[]
                        )
                        new_list.append(nop)
                    inst.sync_info = mybir.SyncInfo(
                        on_wait=waits[:limit], on_update=list(si.on_update)
                    )
                    changed = True
                new_list.append(inst)
            if changed:
                bb.instructions = new_list


def _build():
    if "nc" in _CACHE:
        return _CACHE["nc"]

    _install_tile_drain_patch()

    import concourse.bass as bass
    import concourse.mybir as mybir
    from concourse.tile import TileContext
    from concourse.masks import make_identity

    f32 = mybir.dt.float32
    f16 = mybir.dt.float16
    X = mybir.AxisListType.X

    nc = bass.Bass()

    # ---- DRAM I/O (identical on every core except qg/qd shards) ----
    rfq = nc.dram_tensor("rfq", [KR, BS, S], f32, kind="ExternalInput")
    rfk = nc.dram_tensor("rfk", [KR, BS, S], f32, kind="ExternalInput")
    d_qTF = nc.dram_tensor("d_qTF", [S, BS, DIM], f16, kind="ExternalInput")
    d_kF = nc.dram_tensor("d_kF", [DIM, BS, S], f16, kind="ExternalInput")
    g_qF = nc.dram_tensor("g_qF", [BS, DIM], f32, kind="ExternalInput")
    g_kF = nc.dram_tensor("g_kF", [BS, DIM], f32, kind="ExternalInput")
    g_qT5 = nc.dram_tensor("g_qT5", [DIM, BS], f16, kind="ExternalInput")
    qg = nc.dram_tensor("qg", [DIM, QS], f16, kind="ExternalInput")
    qd = nc.dram_tensor("qd", [DIM, QS], f16, kind="ExternalInput")

    out_d = nc.dram_tensor("out_d", [QS, BS, S], f16, kind="ExternalOutput")
    out_g = nc.dram_tensor("out_g", [BS, QS], f16, kind="ExternalOutput")
    out_pos = nc.dram_tensor("out_pos", [1, BS * S], f32, kind="ExternalOutput")
    out_posg = nc.dram_tensor("out_posg", [BS, 1], f32, kind="ExternalOutput")


    with TileContext(nc) as tc:
        with (
            tc.tile_pool(name="const", bufs=1) as const_pool,
            tc.tile_pool(name="queues", bufs=1) as queue_pool,
            tc.tile_pool(name="dqm", bufs=1) as dqm_pool,
            tc.tile_pool(name="small", bufs=3) as small_pool,
            tc.tile_pool(name="posp", bufs=1) as pos_pool,
            tc.tile_pool(name="stage", bufs=6) as stage_pool,
            tc.tile_pool(name="gstage", bufs=2) as gstage_pool,
        ):
            # ---- constants ----
            ident = const_pool.tile([128, 128], f32)
            make_identity(nc, ident)
            ident16 = const_pool.tile([128, 128], f16)
            nc.vector.tensor_copy(ident16[:], ident[:])
            ones = const_pool.tile([128, 1], f32)
            nc.vector.memset(ones, 1.0)

            # ---- loads: R factors first (they gate the match), then
            # the gather/queue tensors, all on the sync HWDGE ring; d_k
            # (pos only) on the scalar ring. ----
            rfq_sb = const_pool.tile([KR, BS, S], f32)
            nc.sync.dma_start(rfq_sb[:], rfq[:, :, :])
            rfk_sb = const_pool.tile([KR, BS, S], f32)
            nc.sync.dma_start(rfk_sb[:], rfk[:, :, :])
            d_qT_sb = const_pool.tile([S, BS, DIM], f16)
            nc.sync.dma_start(d_qT_sb[:], d_qTF[:, :, :])
            g_q_sb = const_pool.tile([BS, DIM], f32)
            nc.sync.dma_start(g_q_sb[:], g_qF[:, :])
            g_k_sb = const_pool.tile([BS, DIM], f32)
            nc.sync.dma_start(g_k_sb[:], g_kF[:, :])
            g_qT5_sb = const_pool.tile([128, BS], f16)
            nc.sync.dma_start(g_qT5_sb[:], g_qT5[:, :])
            qd_sb = queue_pool.tile([128, QS], f16, tag="qd")
            qg_sb = queue_pool.tile([128, QS], f16, tag="qg")
            for h in range(2):
                sl = slice(h * (QS // 2), (h + 1) * (QS // 2))
                nc.sync.dma_start(qd_sb[:, sl], qd[:, sl])
            for h in range(2):
                sl = slice(h * (QS // 2), (h + 1) * (QS // 2))
                nc.sync.dma_start(qg_sb[:, sl], qg[:, sl])
            d_k_sb = const_pool.tile([128, BS, S], f16)
            nc.scalar.dma_start(d_k_sb[:], d_kF[:, :, :])

            dqm_all = dqm_pool.tile([128, BS * S], f16, tag="dqma")

            p1_psum = tc.tile_pool(name="p1psum", bufs=2, space="PSUM")
            pcos_pool = p1_psum.__enter__()
            poh_psum = tc.tile_pool(name="pohpsum", bufs=2, space="PSUM")
            poh_pool = poh_psum.__enter__()
            pdqm_psum = tc.tile_pool(name="pdqmpsum", bufs=2, space="PSUM")
            pdqm_pool = pdqm_psum.__enter__()

            # ---- match: cosine (one K=98 matmul per batch; 16 batches
            # packed per PSUM bank), then argmax -> one-hot -> gather ----
            with nc.named_scope("p1"):
                pcos_t = []
                for hb in range(2):
                    tile = pcos_pool.tile([128, 8 * S], f32, tag="pcos",
                                          name=f"pcos{hb}")
                    pcos_t.append(tile)
                    for k in range(16):
                        b = 16 * hb + k
                        slot = k // 2
                        h = k % 2
                        s0 = slot * S
                        nc.tensor.matmul(
                            tile[64 * h : 64 * h + S, s0 : s0 + S],
                            rfq_sb[:, b, :],
                            rfk_sb[:, b, :],
                            start=(slot == 0),
                            stop=True,
                            tile_position=(0, 64 * h),
                            skip_group_check=True,
                        )
                for bp in range(16):
                    tile = pcos_t[bp // 8]
                    s0 = (bp % 8) * S
                    csl = tile[:, s0 : s0 + S]           # 2 batches packed
                    cmax = small_pool.tile([128, 1], f32, tag="cmax")
                    nc.vector.reduce_max(out=cmax[:], in_=csl, axis=X)
                    onehT = small_pool.tile([128, S], f16, tag="onehT")
                    nc.vector.tensor_scalar(
                        onehT[:], csl, cmax[:], INV_TAU,
                        mybir.AluOpType.is_ge, mybir.AluOpType.mult,
                    )
                    poh = poh_pool.tile([S, 128], f16, tag="poh")
                    nc.tensor.transpose(poh, onehT[:], ident16[:])
                    oneh = small_pool.tile([S, 128], f16, tag="oneh")
                    nc.scalar.copy(oneh[:], poh[:])
                    pdqm = pdqm_pool.tile([128, 2, S], f32, tag="pdqm")
                    for h in range(2):
                        bi = 2 * bp + h
                        nc.tensor.matmul(
                            pdqm[:, h, :], d_qT_sb[:, bi, :],
                            oneh[:, 64 * h : 64 * h + S],
                            start=(h == 0), stop=True,
                            skip_group_check=True,
                        )
                    dst = dqm_all[:, 2 * bp * S : (2 * bp + 2) * S]
                    if bp % 2 == 0:
                        nc.vector.tensor_copy(
                            dst.rearrange("p (c s) -> p c s", c=2), pdqm[:]
                        )
                    else:
                        nc.scalar.copy(
                            dst.rearrange("p (c s) -> p c s", c=2), pdqm[:]
                        )

            pdqm_psum.__exit__(None, None, None)
            poh_psum.__exit__(None, None, None)
            p1_psum.__exit__(None, None, None)
            pmm_ctx = tc.tile_pool(name="pmm", bufs=3, space="PSUM")
            pmm_pool = pmm_ctx.__enter__()
            pg_ctx = tc.tile_pool(name="pg", bufs=2, space="PSUM")
            pg_pool = pg_ctx.__enter__()

            # ---- out_g = (g_q.T * invtau).T @ queue_g shard, emitted
            # between the match tail and phase 2 so its stores fill the
            # pre-phase-2 DMA gap ----
            with nc.named_scope("gphase"):
                for nt4 in range(QS // 2048):
                    gst = gstage_pool.tile([BS, 4, 512], f16, tag="gstage")
                    for k in range(4):
                        nt = nt4 * 4 + k
                        pg = pg_pool.tile([BS, 512], f32, tag="pg")
                        nc.tensor.matmul(
                            pg,
                            g_qT5_sb[:],
                            qg_sb[:, nt * 512 : (nt + 1) * 512],
                            start=True,
                            stop=True,
                        )
                        if k % 2 == 0:
                            nc.vector.tensor_copy(gst[:, k], pg[:])
                        else:
                            nc.scalar.copy(gst[:, k], pg[:])
                    nc.sync.dma_start(
                        out_g[:, nt4 * 2048 : (nt4 + 1) * 2048],
                        gst[:].rearrange("b k n -> b (k n)"),
                    )

            # ---- pos logits, fused over all batches; emitted mid-p2
            # (off the critical path); the ones-matmuls borrow pmm-pool
            # tiles ----
            def emit_pos():
                prod = pos_pool.tile([128, BS * S], f32, tag="prod",
                                     name="prod")
                nc.vector.tensor_tensor(
                    prod[:],
                    d_k_sb[:].rearrange("p b s -> p (b s)"),
                    dqm_all[:],
                    mybir.AluOpType.mult,
                )
                posrow = pos_pool.tile([1, BS * S], f32, tag="posrow",
                                       name="posrow")
                for i in range(4):
                    sl = slice(i * 8 * S, (i + 1) * 8 * S)
                    ppos = pmm_pool.tile([128, 2, 512], f32, tag="pmm",
                                         name=f"ppos{i}")
                    nc.tensor.matmul(
                        ppos[0:1, 0, : 8 * S], ones[:, :], prod[:, sl],
                        start=True, stop=True,
                    )
                    nc.scalar.copy(posrow[:, sl], ppos[0:1, 0, : 8 * S])
                nc.sync.dma_start(out_pos[:, :], posrow[:])
                prodg = small_pool.tile([BS, DIM], f32, tag="prodg")
                nc.vector.tensor_tensor(
                    prodg[:], g_q_sb[:], g_k_sb[:], mybir.AluOpType.mult
                )
                posg = small_pool.tile([BS, 1], f32, tag="posg")
                nc.vector.reduce_sum(out=posg[:], in_=prodg[:], axis=X)
                posg5 = small_pool.tile([BS, 1], f32, tag="posg5")
                nc.vector.tensor_scalar_mul(posg5[:], posg[:], INV_TAU)
                nc.sync.dma_start(out_posg[:, :], posg5[:])

            # ---- phase 2: out_d over the q shard, all 32 batches per
            # tile; two matmuls share a two-bank PSUM tile so each half
            # needs only one fused PSUM->SBUF copy ----
            with nc.named_scope("p2"):
                for qt in range(QT):
                    stg = stage_pool.tile([128, BS, S], f16, tag="stage")
                    for half in range(2):
                        pmm = pmm_pool.tile([128, 2, 512], f32, tag="pmm")
                        for g2 in range(2):
                            g = 2 * half + g2
                            nc.tensor.matmul(
                                pmm[:, g2, : 8 * S],
                                qd_sb[:, qt * 128 : (qt + 1) * 128],
                                dqm_all[:, g * 8 * S : (g + 1) * 8 * S],
                                start=True,
                                stop=True,
                            )
                        src = pmm[:, :, : 8 * S].rearrange(
                            "p c (b s) -> p c b s", b=8
                        )
                        dst = stg[
                            :, half * 16 : (half + 1) * 16, :
                        ].rearrange("p (c b) s -> p c b s", c=2)
                        if half == 0:
                            nc.vector.tensor_copy(dst, src)
                        else:
                            nc.scalar.copy(dst, src)
                    nc.sync.dma_start(
                        out_d[qt * 128 : (qt + 1) * 128, :, :], stg[:]
                    )
                    if qt == 8:
                        emit_pos()
            pg_ctx.__exit__(None, None, None)
            pmm_ctx.__exit__(None, None, None)


    _split_multi_waits(nc, mybir)

    _CACHE["nc"] = nc
    return nc


def prepare_in_maps(inputs):
    g_q = np.ascontiguousarray(inputs["g_q"], dtype=np.float32)
    g_k = np.ascontiguousarray(inputs["g_k"], dtype=np.float32)
    d_q = np.asarray(inputs["d_q"], dtype=np.float32)
    d_k = np.asarray(inputs["d_k"], dtype=np.float32)
    feat_q = np.asarray(inputs["feat_q"], dtype=np.float64)
    feat_k = np.asarray(inputs["feat_k"], dtype=np.float64)
    queue_g = np.asarray(inputs["queue_g"], dtype=np.float32)
    queue_d = np.asarray(inputs["queue_d"], dtype=np.float32)

    def to_f16(a):
        # The PE mishandles fp16 subnormals in the weight path (NaN
        # products); flush them to zero (|err| <= 6.1e-5, negligible here).
        a = a.astype(np.float16)
        a[np.abs(a) < np.float16(6.104e-5)] = np.float16(0)
        return a

    # Exact low-rank reparameterization of the match features: per
    # batch, [feat_k | feat_q] = Q R with Q orthonormal, so
    # <feat_k_i, feat_q_j> = <R_i, R_j>.
    rfk = np.empty((KR, BS, S), dtype=np.float32)
    rfq = np.empty((KR, BS, S), dtype=np.float32)
    for b in range(BS):
        M = np.concatenate([feat_k[b], feat_q[b]], axis=1)  # [CF, 2S]
        R = np.linalg.qr(M, mode="r")
        rfk[:, b, :] = R[:, :S].astype(np.float32)
        rfq[:, b, :] = R[:, S:].astype(np.float32)

    d_qT = to_f16(np.ascontiguousarray(d_q.transpose(2, 0, 1)))    # [S,BS,DIM]
    d_kX = to_f16(np.ascontiguousarray(d_k.transpose(1, 0, 2)))    # [DIM,BS,S]
    g_qT5 = to_f16(np.ascontiguousarray(g_q.T * np.float32(INV_TAU)))
    qg16 = to_f16(queue_g)
    qd16 = to_f16(queue_d)

    in_maps = []
    for c in range(NCORES):
        sh = slice(c * QS, (c + 1) * QS)
        in_maps.append(
            {
                "rfq": rfq,
                "rfk": rfk,
                "d_qTF": d_qT,
                "d_kF": d_kX,
                "g_qF": g_q,
                "g_kF": g_k,
                "g_qT5": g_qT5,
                "qg": np.ascontiguousarray(qg16[:, sh]),
                "qd": np.ascontiguousarray(qd16[:, sh]),
            }
        )
    return in_maps


def assemble(results) -> np.ndarray:
    BL = BS // NCORES
    out = np.empty((BS, 1 + Q, 1 + S), dtype=np.float32)
    for c in range(NCORES):
        posd = results[c]["out_pos"].reshape(BS, S)
        posg = results[c]["out_posg"].reshape(BS)
        bl = slice(c * BL, (c + 1) * BL)
        out[bl, 0, 1:] = posd[bl]
        out[bl, 0, 0] = posg[bl]
        rows = slice(1 + c * QS, 1 + (c + 1) * QS)
        out[:, rows, 0] = results[c]["out_g"].astype(np.float32)
        out[:, rows, 1:] = (
            results[c]["out_d"].transpose(1, 0, 2).astype(np.float32)
        )
    return out


def kernel(**inputs) -> np.ndarray:
    from concourse.bass_utils import run_bass_kernel_spmd

    nc = _build()
    in_maps = prepare_in_maps(inputs)
    res = run_bass_kernel_spmd(nc, in_maps, core_ids=list(range(NCORES)))
    return assemble(res.results)
